# revision 3
# baseline (speedup 1.0000x reference)
"""IsoMaxPlus first-part kernel for Trainium2 (8 NeuronCores, SPMD).

Math (per point n, prototype k):
    c[n,k] = (x_n . p_hat_k) / ||x_n||
    out[n,k] = -|s| * sqrt(2 - 2 c[n,k])

Device layout (per group g of 2048 points = 2 macro-tiles of 1024):
  - input host-preblocked: feat[g] = [128, 2, 2048] bf16 contiguous 1MB,
    one sync-HWDGE dma per group (128 x 8KB descriptors)
  - PSUM tile A [128, 1024] f32 per group, col-tiled matmuls:
      S(m)=ssq rows at partition 0 / 32 (accumulates ones.T@q1 + ones.T@q2)
      G(m)=W.T@x at partitions 64/96 (accumulates W0.T@x0 + W1.T@x1)
    4 col groups stream concurrently on the PE array.
  - tail: rt=sqrt(S-rows) [33,1024]; compact [128,16]; 1/x; DRAM rid;
    broadcast back to [51,1024] rows 64..114; T=G*R; U=sqrt(-2s^2 T+2s^2)
  - 6-stage software pipeline (lags 1,2,3,4,5) with per-engine queue
    orders chosen so no op head-of-line-blocks its engine:
      sync:   in(i), out(i-5) x2
      vector: T(i-5), q1(i-1), q2a(i-1), recip(i-3)
      scalar: q2b(i-1), U(i-5), rt(i-2)
      gpsimd: R(i-4) x2, cm(i-3) x2, rid(i-3) x2
      tensor: 16 matmuls (i-2)
"""

import numpy as np

B, C, H, W = 16, 256, 128, 256
K = 19
NCORES = 8
BPC = B // NCORES
HW = H * W
NF = 1024
GF = 2 * NF
NGRP = BPC * HW // GF      # 32 groups per core
EPS = 1e-12


def _split_excess_waits(nc):
    """Walrus limits the sync-wait slots per ISA instruction. Hoist excess
    waits onto same-engine NoOps inserted right before the instruction."""
    import bass_rust
    import concourse.mybir as mybir

    limits = {}
    default_limit = 1
    skip = {"InstEventSemaphore", "InstNoOp", "InstCall",
            "InstUnconditionalBranch", "InstISA", "InstRegisterMove"}
    nseq = 0
    for fn in nc.m.functions:
        for blk in fn.blocks:
            new = []
            for I in blk.instructions:
                tn = type(I).__name__
                si = I.sync_info
                waits = list(si.on_wait) if si else []
                lim = limits.get(tn, default_limit)
                if tn in skip or len(waits) <= lim:
                    new.append(I)
                    continue
                keep = waits[-lim:]
                excess = waits[:-lim]
                for w in excess:
                    nop = mybir.InstNoOp(name=f"{I.name}-w{nseq}", ins=[], outs=[])
                    nseq += 1
                    nop.engine = I.engine
                    nop.sync_info = bass_rust.SyncInfo(on_wait=[w], on_update=[])
                    new.append(nop)
                I.sync_info = bass_rust.SyncInfo(
                    on_wait=keep, on_update=list(si.on_update) if si else []
                )
                new.append(I)
            blk.instructions = new
    return nc


def build_program(split_waits=True):
    from contextlib import ExitStack

    import concourse.bass as bass
    import concourse.mybir as mybir
    import concourse.tile as tile

    f32 = mybir.dt.float32
    bf16 = mybir.dt.bfloat16
    AF = mybir.ActivationFunctionType

    nc = bass.Bass()
    feat = nc.declare_dram_parameter("features", [NGRP // 4, 128, 8, GF],
                                     bf16, isOutput=False)
    wp = nc.declare_dram_parameter("wproto", [128, 2, K], bf16, isOutput=False)
    sv = nc.declare_dram_parameter("svec", [128, 1], f32, isOutput=False)
    bv = nc.declare_dram_parameter("bvec", [128, 1], f32, isOutput=False)
    out = nc.declare_dram_parameter("out", [BPC, K, HW], bf16, isOutput=True)
    rid = nc.dram_tensor("ridscratch", (2 * NGRP, NF), bf16, kind="Internal")

    with ExitStack() as ctx:
        tc = ctx.enter_context(tile.TileContext(nc))
        singles = ctx.enter_context(tc.tile_pool(name="singles", bufs=1))
        xpool = ctx.enter_context(tc.tile_pool(name="x", bufs=3))
        qpool = ctx.enter_context(tc.tile_pool(name="q", bufs=6))
        apool = ctx.enter_context(tc.tile_pool(name="a", bufs=3, space="PSUM"))
        cpool = ctx.enter_context(tc.tile_pool(name="c", bufs=8))
        rpool = ctx.enter_context(tc.tile_pool(name="r", bufs=4))
        tpool = ctx.enter_context(tc.tile_pool(name="t", bufs=3))
        opool = ctx.enter_context(tc.tile_pool(name="o", bufs=3))
        gcpool = ctx.enter_context(tc.tile_pool(name="gc", bufs=7))

        w_s = singles.tile([128, 2, K], bf16)
        nc.sync.dma_start(out=w_s, in_=wp[:, :, :])
        ones_s = singles.tile([128, 1], bf16)
        nc.vector.memset(ones_s, 1.0)
        sv_s = singles.tile([128, 1], f32)
        nc.sync.dma_start(out=sv_s, in_=sv[:, :])
        bv_s = singles.tile([128, 1], f32)
        nc.sync.dma_start(out=bv_s, in_=bv[:, :])

        S = {"xt": {}, "q1": {}, "q2": {}, "A": {}, "rt": {}, "ic": {},
             "R": {}, "T": {}, "U": {}}

        def op_load4(g4):
            xt = xpool.tile([128, 8, GF], bf16, tag="xt")
            nc.sync.dma_start(out=xt, in_=feat[g4, :, :, :])
            for k in range(4):
                S["xt"][4 * g4 + k] = xt

        def op_T(g):
            T = tpool.tile([115, NF], bf16, tag="t")
            Gc, R = S["Gc"].pop(g), S["R"].pop(g)
            nc.vector.tensor_mul(
                out=T[64:115, :], in0=Gc[64:115, :], in1=R[64:115, :]
            )
            S["T"][g] = T

        def op_q2b(g):
            xt = S["xt"][g][:, 2 * (g % 4):2 * (g % 4) + 2]
            q2 = qpool.tile([128, GF], bf16, tag="q2")
            nc.scalar.activation(
                out=q2[:, NF:GF], in_=xt[:, 1, NF:GF], func=AF.Square
            )
            S["q2"][g] = q2

        def op_U(g):
            T = S["T"].pop(g)
            Ut = opool.tile([115, NF], bf16, tag="u")
            nc.scalar.activation(
                out=Ut[64:115, :], in_=T[64:115, :], func=AF.Sqrt,
                bias=bv_s[64:115, :], scale=sv_s[64:115, :],
            )
            S["U"][g] = Ut

        def op_out(g):
            b, h0 = g >> 4, (g & 15) * GF
            Ut = S["U"].pop(g)
            for j in range(2):
                nc.sync.dma_start(
                    out=out[b, :, h0 + j * NF:h0 + (j + 1) * NF],
                    in_=Ut[64 + 32 * j:64 + 32 * j + K, :],
                )

        def op_q1(g):
            xt = S["xt"][g][:, 2 * (g % 4):2 * (g % 4) + 2]
            q1 = qpool.tile([128, GF], bf16, tag="q1")
            nc.vector.tensor_mul(out=q1, in0=xt[:, 0, :], in1=xt[:, 0, :])
            S["q1"][g] = q1
            q2 = S["q2"][g]
            nc.vector.tensor_mul(
                out=q2[:, 0:NF], in0=xt[:, 1, 0:NF], in1=xt[:, 1, 0:NF]
            )

        def op_R(g):
            R = rpool.tile([128, NF], bf16, tag="rb")
            nc.gpsimd.dma_start(
                out=R[64:128, :],
                in_=rid.ap()[2 * g:2 * g + 2, :]
                .partition_broadcast(32).transpose([1, 0, 2]),
            )
            S["R"][g] = R

        def op_mm(g):
            xt = S["xt"].pop(g)[:, 2 * (g % 4):2 * (g % 4) + 2]
            q1, q2 = S["q1"].pop(g), S["q2"].pop(g)
            A = apool.tile([128, NF], f32)
            for si in range(2):
                sl = slice(si * 512, (si + 1) * 512)
                for ph in range(2):  # phase 0: c0/q1, phase 1: c1/q2
                    q = (q1, q2)[ph]
                    for j in range(2):
                        nc.tensor.matmul(
                            out=A[64 + 32 * j:64 + 32 * j + K, sl],
                            lhsT=w_s[:, ph, :],
                            rhs=xt[:, ph, j * NF + si * 512:
                                   j * NF + (si + 1) * 512],
                            start=(ph == 0), stop=(ph == 1),
                            tile_position=(0, 64 + 32 * j),
                        )
                    for j in range(2):
                        nc.tensor.matmul(
                            out=A[32 * j:32 * j + 1, sl],
                            lhsT=ones_s,
                            rhs=q[:, j * NF + si * 512:j * NF + (si + 1) * 512],
                            start=(ph == 0), stop=(ph == 1),
                            tile_position=(0, 32 * j),
                        )
            S["A"][g] = A

        def op_rt(g):
            A = S["A"].pop(g)
            rt = cpool.tile([33, NF], bf16, tag="rt")
            nc.scalar.activation(out=rt, in_=A[0:33, :], func=AF.Sqrt)
            S["rt"][g] = rt
            Gc = gcpool.tile([115, NF], bf16, tag="gc")
            nc.scalar.copy(out=Gc[64:115, :], in_=A[64:115, :])
            S["Gc"] = S.get("Gc") or {}
            S["Gc"][g] = Gc

        def op_cm(g):
            rt = S["rt"].pop(g)
            cm = cpool.tile([128, 16], bf16, tag="cm")
            nc.gpsimd.dma_start(out=cm, in_=rt[0:33:32, :])
            S["ic"][g] = cm

        def op_recip(g):
            cm = S["ic"][g]
            ic = cpool.tile([128, 16], bf16, tag="ic")
            with nc.allow_low_precision(reason="bf16 ok: 2e-2 rel tol"):
                nc.vector.reciprocal(out=ic, in_=cm)
            S["ic"][g] = ic

        def op_rid(g):
            ic = S["ic"].pop(g)
            nc.gpsimd.dma_start(out=rid.ap()[2 * g:2 * g + 2, :], in_=ic)

        # lags: load 0 | squares 1 | mm+rt 2 | cm/recip/rid 3 | R 4 | tail 5
        # per-engine queue order: bulk ops lead, chain ops trail
        for i in range(NGRP + 8):
            if i % 4 == 0 and (i + 4) // 4 < NGRP // 4:
                op_load4((i + 4) // 4)
            if i == 0:
                op_load4(0)
            if 0 <= i - 1 < NGRP:
                op_q2b(i - 1)
                op_q1(i - 1)
            if 0 <= i - 5 < NGRP:
                op_R(i - 5)
            if 0 <= i - 2 < NGRP:
                op_mm(i - 2)
                op_rt(i - 2)
            if 0 <= i - 7 < NGRP:
                op_T(i - 7)
                op_U(i - 7)
                op_out(i - 7)
            if 0 <= i - 3 < NGRP:
                op_cm(i - 3)
                op_recip(i - 3)
                op_rid(i - 3)

    return _split_excess_waits(nc) if split_waits else nc


def host_inputs(features, prototypes, distance_scale):
    """Build per-core input maps (host-side prep)."""
    import ml_dtypes

    pn = prototypes / np.maximum(
        np.sqrt(np.sum(prototypes * prototypes, axis=-1, keepdims=True)), EPS
    )
    s = abs(float(np.asarray(distance_scale).reshape(-1)[0]))
    wproto = np.ascontiguousarray(
        pn.T.reshape(2, 128, K).transpose(1, 0, 2)
    ).astype(ml_dtypes.bfloat16)
    svec = np.full((128, 1), -2.0 * s * s, np.float32)
    bvec = np.full((128, 1), 2.0 * s * s, np.float32)

    fr = (
        features.reshape(NCORES, BPC, 2, 128, HW // (4 * GF), 4, GF)
        .astype(ml_dtypes.bfloat16)
        .transpose(0, 1, 4, 3, 5, 2, 6)
    )
    in_maps = []
    for i in range(NCORES):
        in_maps.append(
            {
                "features": np.ascontiguousarray(fr[i]).reshape(
                    NGRP // 4, 128, 8, GF
                ),
                "wproto": wproto,
                "svec": svec,
                "bvec": bvec,
            }
        )
    return in_maps


_CACHE = {}


def kernel(features, prototypes, distance_scale):
    from concourse.bass_utils import run_bass_kernel_spmd

    if "nc" not in _CACHE:
        _CACHE["nc"] = build_program()
    nc = _CACHE["nc"]
    in_maps = host_inputs(features, prototypes, distance_scale)
    res = run_bass_kernel_spmd(nc, in_maps, core_ids=list(range(NCORES)))
    out = np.empty((NCORES, BPC, K, H, W), np.float32)
    for i in range(NCORES):
        np.multiply(
            res.results[i]["out"].reshape(BPC, K, H, W).astype(np.float32),
            -1.0,
            out=out[i],
        )
    return out.reshape(B, K, H, W)
